# revision 19
# baseline (speedup 1.0000x reference)
"""AttentionalCorrelation kernel for 8 trn2 NeuronCores.

Per-batch computation (B=8 batches, one per core):
    q = wq @ f1 + bq            [64, 4096]
    k = wk @ f2 + bk            [64, 4096]
    logits = q^T k              [4096, 4096]
    attn = softmax(logits, -1)
    out = wf @ (f2 @ attn^T) + bf   [64, 4096]

Kernel strategy (per core):
  - S^T tiles [m=128, n=512] = k_tile^T @ q  (m on partitions) so the softmax
    denominator direction (m) is the matmul contraction direction downstream.
  - G = (wf @ f2)^T [4096, 64] precomputed, augmented with a ones column; the
    PV matmul  out'[o,n] = sum_m G_aug[m,o] * exp(S^T[m,n])  then yields the
    projected output AND the softmax denominator (row 64) in one accumulation.
  - exp is unnormalized (no max subtraction: |logits| <~ 15, safe in fp32);
    final tiles are scaled by 1/denominator and biased.
  - q/k are duplicated into both partition halves so the QK matmul contracts
    over K=128 (computing 2*S, fixed by exp(scale=0.5)).
"""

import numpy as np

import concourse.bacc as bacc
import concourse.tile as tile
from concourse import mybir
from concourse import bass_utils

F32 = mybir.dt.float32
F32R = mybir.dt.float32r
F16 = mybir.dt.float16
EXP = mybir.ActivationFunctionType.Exp

B = 8
C = 128          # input channels
N = 4096         # H*W
HID = 64
OUT = 64
NB, BN = 8, 512  # n blocks
MT, BM = 32, 128  # m tiles

# matmul dtype for the two big matmul stages (QK^T and PV).
# float16 streams 1 col/cycle on the PE (vs 2 for float32r, 4 for float32)
# and has 8x finer mantissa than bfloat16. Unnormalized exp values are
# shifted by EXP_SHIFT so they stay within fp16 range (cancels in the
# normalization).
MM_DT = F16
EXP_SHIFT = -8.0


def ts(i, size):
    return slice(i * size, (i + 1) * size)


def _emit(nc, tc, io):
    with tc.tile_pool(name="consts", bufs=1) as consts:
        _emit_inner(nc, tc, io, consts)


def _emit_inner(nc, tc, io, consts):
    f1, f2, wqT, wkT, wfT, bq, bk, bf, out = io

    F1 = consts.tile([C, N], F32)
    F2 = consts.tile([C, N], F32)
    F1H = consts.tile([C, N], MM_DT)
    F2H = consts.tile([C, N], MM_DT)
    WQT = consts.tile([C, HID], F32)
    nc.sync.dma_start(WQT[:], wqT[:])
    WKT = consts.tile([C, HID], F32)
    nc.sync.dma_start(WKT[:], wkT[:])
    WFT = consts.tile([C, OUT], F32)
    nc.sync.dma_start(WFT[:], wfT[:])
    WQT16 = consts.tile([C, HID], MM_DT)
    nc.vector.tensor_copy(WQT16[:], WQT[:])
    WKT16 = consts.tile([C, HID], MM_DT)
    nc.vector.tensor_copy(WKT16[:], WKT[:])
    WFT16 = consts.tile([C, OUT], MM_DT)
    nc.vector.tensor_copy(WFT16[:], WFT[:])
    BQ2 = consts.tile([128, 1], F32)
    nc.sync.dma_start(BQ2[0:HID, :], bq[:])
    nc.sync.dma_start(BQ2[HID:128, :], bq[:])
    BK2 = consts.tile([128, 1], F32)
    nc.sync.dma_start(BK2[0:HID, :], bk[:])
    nc.sync.dma_start(BK2[HID:128, :], bk[:])
    BF = consts.tile([OUT, 1], F32)
    nc.sync.dma_start(BF[:], bf[:])

    Q2 = consts.tile([128, N], MM_DT)   # q duplicated in both partition halves
    K2 = consts.tile([128, N], MM_DT)
    GA = consts.tile([128, MT, OUT + 1], MM_DT)  # G tiles + ones column

    ONES = consts.tile([128, MT], F32)
    nc.vector.memset(ONES[:], 1.0)
    nc.vector.tensor_copy(GA[:, :, OUT:].squeeze(), ONES[:])
    ESHIFT = consts.tile([128, 1], F32)
    nc.vector.memset(ESHIFT[:], EXP_SHIFT)

    # Phase-1 matmuls share the stp PSUM slots (8 banks total: stp 3x2 +
    # pvp 2x1) so phase 1 overlaps the main loop start without starving the
    # exp pipeline of st buffers.
    with (
        tc.tile_pool(name="stp", bufs=3, space="PSUM") as stp,
        tc.tile_pool(name="pvp", bufs=2, space="PSUM") as pvp,
        tc.tile_pool(name="etp", bufs=4) as etp,
        tc.tile_pool(name="finp", bufs=3) as finp,
    ):
        def emit_q(j):
            # q written to both partition halves via column-tiled matmul
            # pair (no SBUF->SBUF duplication DMA).
            blk = ts(j, BN)
            nc.gpsimd.dma_start(F1[:, blk], f1[:, blk])
            nc.vector.tensor_copy(F1H[:, blk], F1[:, blk])
            pq = stp.tile([BM, 2 * BN], F32, tag="st")
            nc.tensor.matmul(pq[0:HID, 0:BN], lhsT=WQT16[:], rhs=F1H[:, blk],
                             start=True, stop=True, tile_position=(0, 0))
            nc.tensor.matmul(pq[HID:128, 0:BN], lhsT=WQT16[:], rhs=F1H[:, blk],
                             start=True, stop=True, tile_position=(0, HID))
            nc.vector.tensor_scalar_add(Q2[:, blk], pq[:, 0:BN], BQ2[:])

        # ---- phase 1: q0 first (main-loop critical path), then k and G ----
        emit_q(0)
        for j in range(NB):
            blk = ts(j, BN)
            nc.sync.dma_start(F2[:, blk], f2[:, blk])
            nc.vector.tensor_copy(F2H[:, blk], F2[:, blk])
            pk = stp.tile([BM, 2 * BN], F32, tag="st")
            nc.tensor.matmul(pk[0:HID, 0:BN], lhsT=WKT16[:], rhs=F2H[:, blk],
                             start=True, stop=True, tile_position=(0, 0))
            nc.tensor.matmul(pk[HID:128, 0:BN], lhsT=WKT16[:], rhs=F2H[:, blk],
                             start=True, stop=True, tile_position=(0, HID))
            nc.vector.tensor_scalar_add(K2[:, blk], pk[:, 0:BN], BK2[:])
            pg = stp.tile([BM, 2 * BN], F32, tag="st")
            for i, t in enumerate(range(4 * j, 4 * j + 4)):
                nc.tensor.matmul(pg[:, ts(i, OUT)], lhsT=F2H[:, ts(t, BM)],
                                 rhs=WFT16[:], start=True, stop=True)
            nc.vector.tensor_copy(
                GA[:, 4 * j:4 * j + 4, 0:OUT],
                pg[:, 0:4 * OUT].rearrange("p (g o) -> p g o", g=4),
            )

        emit_q(1)

        # ---- main loop ----
        for j in range(NB):
            if j + 2 < NB:
                emit_q(j + 2)
            pv = pvp.tile([OUT + 1, BN], F32, tag="pv")
            for t2 in range(MT // 2):
                st = stp.tile([BM, 2 * BN], F32, tag="st")
                nc.tensor.matmul(st[:, 0:BN], lhsT=K2[:, ts(2 * t2, BM)],
                                 rhs=Q2[:, ts(j, BN)],
                                 start=True, stop=True)
                nc.tensor.matmul(st[:, BN:2 * BN], lhsT=K2[:, ts(2 * t2 + 1, BM)],
                                 rhs=Q2[:, ts(j, BN)],
                                 start=True, stop=True)
                et = etp.tile([BM, 2 * BN], MM_DT, tag="et")
                nc.scalar.activation(et[:], st[:], EXP, scale=0.5,
                                     bias=ESHIFT[:])
                nc.tensor.matmul(pv[:], lhsT=GA[:, 2 * t2, :],
                                 rhs=et[:, 0:BN],
                                 start=(t2 == 0), stop=False)
                nc.tensor.matmul(pv[:], lhsT=GA[:, 2 * t2 + 1, :],
                                 rhs=et[:, BN:2 * BN],
                                 start=False, stop=(t2 == MT // 2 - 1))
            rs = finp.tile([1, BN], F32, tag="rs")
            nc.vector.tensor_copy(rs[:], pv[OUT:, :])
            recip = finp.tile([1, BN], F32, tag="recip")
            nc.vector.reciprocal_approx_fast(recip[:], rs[:])
            recipB = finp.tile([OUT, BN], F32, tag="recipB")
            nc.gpsimd.partition_broadcast(recipB[:], recip[:])
            ob = finp.tile([OUT, BN], F32, tag="ob")
            nc.vector.tensor_mul(ob[:], pv[0:OUT, :], recipB[:])
            nc.vector.tensor_scalar_add(ob[:], ob[:], BF[:])
            nc.gpsimd.dma_start(out[:, ts(j, BN)], ob[:])


_CACHE = {}


def _build():
    if "nc" in _CACHE:
        return _CACHE["nc"]
    nc = bacc.Bacc(None, target_bir_lowering=False, debug=False)
    f1 = nc.dram_tensor("f1", [C, N], F32, kind="ExternalInput")
    f2 = nc.dram_tensor("f2", [C, N], F32, kind="ExternalInput")
    wqT = nc.dram_tensor("wqT", [C, HID], F32, kind="ExternalInput")
    wkT = nc.dram_tensor("wkT", [C, HID], F32, kind="ExternalInput")
    wfT = nc.dram_tensor("wfT", [C, OUT], F32, kind="ExternalInput")
    bq = nc.dram_tensor("bq", [HID, 1], F32, kind="ExternalInput")
    bk = nc.dram_tensor("bk", [HID, 1], F32, kind="ExternalInput")
    bf = nc.dram_tensor("bf", [OUT, 1], F32, kind="ExternalInput")
    out = nc.dram_tensor("out", [OUT, N], F32, kind="ExternalOutput")
    with tile.TileContext(nc) as tc:
        _emit(nc, tc, (f1[:], f2[:], wqT[:], wkT[:], wfT[:],
                       bq[:], bk[:], bf[:], out[:]))
    nc.compile()
    _CACHE["nc"] = nc
    return nc


def _run(inputs, trace=False):
    nc = _build()
    f1 = np.ascontiguousarray(np.asarray(inputs["f1"], dtype=np.float32))
    f2 = np.ascontiguousarray(np.asarray(inputs["f2"], dtype=np.float32))
    shared = {
        "wqT": np.ascontiguousarray(np.asarray(inputs["wq"], np.float32).T),
        "wkT": np.ascontiguousarray(np.asarray(inputs["wk"], np.float32).T),
        "wfT": np.ascontiguousarray(np.asarray(inputs["wf"], np.float32).T),
        "bq": np.asarray(inputs["bq"], np.float32).reshape(HID, 1).copy(),
        "bk": np.asarray(inputs["bk"], np.float32).reshape(HID, 1).copy(),
        "bf": np.asarray(inputs["bf"], np.float32).reshape(OUT, 1).copy(),
    }
    in_maps = [
        {"f1": f1[b].reshape(C, N), "f2": f2[b].reshape(C, N), **shared}
        for b in range(B)
    ]
    res = bass_utils.run_bass_kernel_spmd(
        nc, in_maps, core_ids=list(range(B)), trace=trace
    )
    out = np.stack([res.results[b]["out"] for b in range(B)])
    H = int(np.sqrt(N))
    return out.reshape(B, OUT, H, H), res


def kernel(**inputs):
    out, _ = _run(inputs, trace=False)
    return out


# revision 20
# speedup vs baseline: 1.0511x; 1.0511x over previous
"""AttentionalCorrelation kernel for 8 trn2 NeuronCores.

Per-batch computation (B=8 batches, one per core):
    q = wq @ f1 + bq            [64, 4096]
    k = wk @ f2 + bk            [64, 4096]
    logits = q^T k              [4096, 4096]
    attn = softmax(logits, -1)
    out = wf @ (f2 @ attn^T) + bf   [64, 4096]

Kernel strategy (per core):
  - S^T tiles [m=128, n=512] = k_tile^T @ q  (m on partitions) so the softmax
    denominator direction (m) is the matmul contraction direction downstream.
  - G = (wf @ f2)^T [4096, 64] precomputed, augmented with a ones column; the
    PV matmul  out'[o,n] = sum_m G_aug[m,o] * exp(S^T[m,n])  then yields the
    projected output AND the softmax denominator (row 64) in one accumulation.
  - exp is unnormalized (no max subtraction: |logits| <~ 15, safe in fp32);
    final tiles are scaled by 1/denominator and biased.
  - q/k are duplicated into both partition halves so the QK matmul contracts
    over K=128 (computing 2*S, fixed by exp(scale=0.5)).
"""

import numpy as np

import concourse.bacc as bacc
import concourse.tile as tile
from concourse import mybir
from concourse import bass_utils

F32 = mybir.dt.float32
F32R = mybir.dt.float32r
F16 = mybir.dt.float16
EXP = mybir.ActivationFunctionType.Exp

B = 8
C = 128          # input channels
N = 4096         # H*W
HID = 64
OUT = 64
NB, BN = 8, 512  # n blocks
MT, BM = 32, 128  # m tiles

# matmul dtype for the two big matmul stages (QK^T and PV).
# float16 streams 1 col/cycle on the PE (vs 2 for float32r, 4 for float32)
# and has 8x finer mantissa than bfloat16. Unnormalized exp values are
# shifted by EXP_SHIFT so they stay within fp16 range (cancels in the
# normalization).
MM_DT = F16
EXP_SHIFT = -8.0


def ts(i, size):
    return slice(i * size, (i + 1) * size)


def _emit(nc, tc, io):
    with tc.tile_pool(name="consts", bufs=1) as consts:
        _emit_inner(nc, tc, io, consts)


def _emit_inner(nc, tc, io, consts):
    f1, f2, wqT, wkT, wfT, bq, bk, bf, out = io

    F1H = consts.tile([C, N], MM_DT)
    F2H = consts.tile([C, N], MM_DT)
    WQT16 = consts.tile([C, HID], MM_DT)
    nc.scalar.dma_start(WQT16[:], wqT[:])
    WKT16 = consts.tile([C, HID], MM_DT)
    nc.scalar.dma_start(WKT16[:], wkT[:])
    WFT16 = consts.tile([C, OUT], MM_DT)
    nc.scalar.dma_start(WFT16[:], wfT[:])
    BQ2 = consts.tile([128, 1], F32)
    nc.scalar.dma_start(BQ2[0:HID, :], bq[:])
    nc.scalar.dma_start(BQ2[HID:128, :], bq[:])
    BK2 = consts.tile([128, 1], F32)
    nc.scalar.dma_start(BK2[0:HID, :], bk[:])
    nc.scalar.dma_start(BK2[HID:128, :], bk[:])
    BF = consts.tile([OUT, 1], F32)
    nc.scalar.dma_start(BF[:], bf[:])

    Q2 = consts.tile([128, N], MM_DT)   # q duplicated in both partition halves
    K2 = consts.tile([128, N], MM_DT)
    GA = consts.tile([128, MT, OUT + 1], MM_DT)  # G tiles + ones column

    ONES = consts.tile([128, MT], F32)
    nc.vector.memset(ONES[:], 1.0)
    nc.vector.tensor_copy(GA[:, :, OUT:].squeeze(), ONES[:])
    ESHIFT = consts.tile([128, 1], F32)
    nc.vector.memset(ESHIFT[:], EXP_SHIFT)

    # Phase-1 matmuls share the stp PSUM slots (8 banks total: stp 3x2 +
    # pvp 2x1) so phase 1 overlaps the main loop start without starving the
    # exp pipeline of st buffers.
    with (
        tc.tile_pool(name="stp", bufs=3, space="PSUM") as stp,
        tc.tile_pool(name="pvp", bufs=2, space="PSUM") as pvp,
        tc.tile_pool(name="etp", bufs=4) as etp,
        tc.tile_pool(name="finp", bufs=3) as finp,
    ):
        def emit_q(j):
            # q written to both partition halves via column-tiled matmul
            # pair (no SBUF->SBUF duplication DMA).
            blk = ts(j, BN)
            nc.gpsimd.dma_start(F1H[:, blk], f1[:, blk])
            pq = stp.tile([BM, 2 * BN], F32, tag="st")
            nc.tensor.matmul(pq[0:HID, 0:BN], lhsT=WQT16[:], rhs=F1H[:, blk],
                             start=True, stop=True, tile_position=(0, 0))
            nc.tensor.matmul(pq[HID:128, 0:BN], lhsT=WQT16[:], rhs=F1H[:, blk],
                             start=True, stop=True, tile_position=(0, HID))
            nc.vector.tensor_scalar_add(Q2[:, blk], pq[:, 0:BN], BQ2[:])

        # ---- phase 1: q0 first (main-loop critical path), then k and G ----
        emit_q(0)
        for j in range(NB):
            blk = ts(j, BN)
            nc.sync.dma_start(F2H[:, blk], f2[:, blk])
            pk = stp.tile([BM, 2 * BN], F32, tag="st")
            nc.tensor.matmul(pk[0:HID, 0:BN], lhsT=WKT16[:], rhs=F2H[:, blk],
                             start=True, stop=True, tile_position=(0, 0))
            nc.tensor.matmul(pk[HID:128, 0:BN], lhsT=WKT16[:], rhs=F2H[:, blk],
                             start=True, stop=True, tile_position=(0, HID))
            nc.vector.tensor_scalar_add(K2[:, blk], pk[:, 0:BN], BK2[:])
            pg = stp.tile([BM, 2 * BN], F32, tag="st")
            for i, t in enumerate(range(4 * j, 4 * j + 4)):
                nc.tensor.matmul(pg[:, ts(i, OUT)], lhsT=F2H[:, ts(t, BM)],
                                 rhs=WFT16[:], start=True, stop=True)
            nc.vector.tensor_copy(
                GA[:, 4 * j:4 * j + 4, 0:OUT],
                pg[:, 0:4 * OUT].rearrange("p (g o) -> p g o", g=4),
            )

        emit_q(1)

        # ---- main loop ----
        for j in range(NB):
            if j + 2 < NB:
                emit_q(j + 2)
            pv = pvp.tile([OUT + 1, BN], F32, tag="pv")
            for t2 in range(MT // 2):
                st = stp.tile([BM, 2 * BN], F32, tag="st")
                nc.tensor.matmul(st[:, 0:BN], lhsT=K2[:, ts(2 * t2, BM)],
                                 rhs=Q2[:, ts(j, BN)],
                                 start=True, stop=True)
                nc.tensor.matmul(st[:, BN:2 * BN], lhsT=K2[:, ts(2 * t2 + 1, BM)],
                                 rhs=Q2[:, ts(j, BN)],
                                 start=True, stop=True)
                et = etp.tile([BM, 2 * BN], MM_DT, tag="et")
                nc.scalar.activation(et[:], st[:], EXP, scale=0.5,
                                     bias=ESHIFT[:])
                nc.tensor.matmul(pv[:], lhsT=GA[:, 2 * t2, :],
                                 rhs=et[:, 0:BN],
                                 start=(t2 == 0), stop=False)
                nc.tensor.matmul(pv[:], lhsT=GA[:, 2 * t2 + 1, :],
                                 rhs=et[:, BN:2 * BN],
                                 start=False, stop=(t2 == MT // 2 - 1))
            rs = finp.tile([1, BN], F32, tag="rs")
            nc.vector.tensor_copy(rs[:], pv[OUT:, :])
            recip = finp.tile([1, BN], F32, tag="recip")
            nc.vector.reciprocal_approx_fast(recip[:], rs[:])
            recipB = finp.tile([OUT, BN], F32, tag="recipB")
            nc.gpsimd.partition_broadcast(recipB[:], recip[:])
            ob = finp.tile([OUT, BN], F32, tag="ob")
            nc.vector.tensor_mul(ob[:], pv[0:OUT, :], recipB[:])
            nc.vector.tensor_scalar_add(ob[:], ob[:], BF[:])
            nc.gpsimd.dma_start(out[:, ts(j, BN)], ob[:])


_CACHE = {}


def _build():
    if "nc" in _CACHE:
        return _CACHE["nc"]
    nc = bacc.Bacc(None, target_bir_lowering=False, debug=False)
    f1 = nc.dram_tensor("f1", [C, N], MM_DT, kind="ExternalInput")
    f2 = nc.dram_tensor("f2", [C, N], MM_DT, kind="ExternalInput")
    wqT = nc.dram_tensor("wqT", [C, HID], MM_DT, kind="ExternalInput")
    wkT = nc.dram_tensor("wkT", [C, HID], MM_DT, kind="ExternalInput")
    wfT = nc.dram_tensor("wfT", [C, OUT], MM_DT, kind="ExternalInput")
    bq = nc.dram_tensor("bq", [HID, 1], F32, kind="ExternalInput")
    bk = nc.dram_tensor("bk", [HID, 1], F32, kind="ExternalInput")
    bf = nc.dram_tensor("bf", [OUT, 1], F32, kind="ExternalInput")
    out = nc.dram_tensor("out", [OUT, N], F32, kind="ExternalOutput")
    with tile.TileContext(nc) as tc:
        _emit(nc, tc, (f1[:], f2[:], wqT[:], wkT[:], wfT[:],
                       bq[:], bk[:], bf[:], out[:]))
    nc.compile()
    _CACHE["nc"] = nc
    return nc


def _run(inputs, trace=False):
    nc = _build()
    f1 = np.ascontiguousarray(np.asarray(inputs["f1"], np.float32).astype(np.float16))
    f2 = np.ascontiguousarray(np.asarray(inputs["f2"], np.float32).astype(np.float16))
    shared = {
        "wqT": np.ascontiguousarray(np.asarray(inputs["wq"], np.float32).T.astype(np.float16)),
        "wkT": np.ascontiguousarray(np.asarray(inputs["wk"], np.float32).T.astype(np.float16)),
        "wfT": np.ascontiguousarray(np.asarray(inputs["wf"], np.float32).T.astype(np.float16)),
        "bq": np.asarray(inputs["bq"], np.float32).reshape(HID, 1).copy(),
        "bk": np.asarray(inputs["bk"], np.float32).reshape(HID, 1).copy(),
        "bf": np.asarray(inputs["bf"], np.float32).reshape(OUT, 1).copy(),
    }
    in_maps = [
        {"f1": f1[b].reshape(C, N), "f2": f2[b].reshape(C, N), **shared}
        for b in range(B)
    ]
    res = bass_utils.run_bass_kernel_spmd(
        nc, in_maps, core_ids=list(range(B)), trace=trace
    )
    out = np.stack([res.results[b]["out"] for b in range(B)])
    H = int(np.sqrt(N))
    return out.reshape(B, OUT, H, H), res


def kernel(**inputs):
    out, _ = _run(inputs, trace=False)
    return out


# revision 21
# speedup vs baseline: 1.0586x; 1.0071x over previous
"""AttentionalCorrelation kernel for 8 trn2 NeuronCores.

Per-batch computation (B=8 batches, one per core):
    q = wq @ f1 + bq            [64, 4096]
    k = wk @ f2 + bk            [64, 4096]
    logits = q^T k              [4096, 4096]
    attn = softmax(logits, -1)
    out = wf @ (f2 @ attn^T) + bf   [64, 4096]

Kernel strategy (per core):
  - S^T tiles [m=128, n=512] = k_tile^T @ q  (m on partitions) so the softmax
    denominator direction (m) is the matmul contraction direction downstream.
  - G = (wf @ f2)^T [4096, 64] precomputed, augmented with a ones column; the
    PV matmul  out'[o,n] = sum_m G_aug[m,o] * exp(S^T[m,n])  then yields the
    projected output AND the softmax denominator (row 64) in one accumulation.
  - exp is unnormalized (no max subtraction: |logits| <~ 15, safe in fp32);
    final tiles are scaled by 1/denominator and biased.
  - q/k are duplicated into both partition halves so the QK matmul contracts
    over K=128 (computing 2*S, fixed by exp(scale=0.5)).
"""

import numpy as np

import concourse.bacc as bacc
import concourse.tile as tile
from concourse import mybir
from concourse import bass_utils

F32 = mybir.dt.float32
F32R = mybir.dt.float32r
F16 = mybir.dt.float16
EXP = mybir.ActivationFunctionType.Exp

B = 8
C = 128          # input channels
N = 4096         # H*W
HID = 64
OUT = 64
NB, BN = 8, 512  # n blocks
MT, BM = 32, 128  # m tiles

# matmul dtype for the two big matmul stages (QK^T and PV).
# float16 streams 1 col/cycle on the PE (vs 2 for float32r, 4 for float32)
# and has 8x finer mantissa than bfloat16. Unnormalized exp values are
# shifted by EXP_SHIFT so they stay within fp16 range (cancels in the
# normalization).
MM_DT = F16
EXP_SHIFT = -8.0


def ts(i, size):
    return slice(i * size, (i + 1) * size)


def _emit(nc, tc, io):
    with tc.tile_pool(name="consts", bufs=1) as consts:
        _emit_inner(nc, tc, io, consts)


def _emit_inner(nc, tc, io, consts):
    f1, f2, wqT, wkT, wfT, bq, bk, bf, out = io

    F1H = consts.tile([C, N], MM_DT)
    F2H = consts.tile([C, N], MM_DT)
    WQT16 = consts.tile([C, HID], MM_DT)
    nc.scalar.dma_start(WQT16[:], wqT[:])
    WKT16 = consts.tile([C, HID], MM_DT)
    nc.scalar.dma_start(WKT16[:], wkT[:])
    WFT16 = consts.tile([C, OUT], MM_DT)
    nc.scalar.dma_start(WFT16[:], wfT[:])
    BQ2 = consts.tile([128, 1], F32)
    nc.scalar.dma_start(BQ2[0:HID, :], bq[:])
    nc.scalar.dma_start(BQ2[HID:128, :], bq[:])
    BK2 = consts.tile([128, 1], F32)
    nc.scalar.dma_start(BK2[0:HID, :], bk[:])
    nc.scalar.dma_start(BK2[HID:128, :], bk[:])
    BF = consts.tile([OUT, 1], F32)
    nc.scalar.dma_start(BF[:], bf[:])

    Q2 = consts.tile([128, N], MM_DT)   # q duplicated in both partition halves
    K2 = consts.tile([128, N], MM_DT)
    GA = consts.tile([128, MT, OUT + 1], MM_DT)  # G tiles + ones column

    ONES = consts.tile([128, MT], F32)
    nc.vector.memset(ONES[:], 1.0)
    nc.vector.tensor_copy(GA[:, :, OUT:].squeeze(), ONES[:])
    ESHIFT = consts.tile([128, 1], F32)
    nc.vector.memset(ESHIFT[:], EXP_SHIFT)

    # Phase-1 matmuls share the stp PSUM slots (8 banks total: stp 3x2 +
    # pvp 2x1) so phase 1 overlaps the main loop start without starving the
    # exp pipeline of st buffers.
    with (
        tc.tile_pool(name="stp", bufs=3, space="PSUM") as stp,
        tc.tile_pool(name="pvp", bufs=2, space="PSUM") as pvp,
        tc.tile_pool(name="etp", bufs=4) as etp,
        tc.tile_pool(name="finp", bufs=3) as finp,
    ):
        def emit_q(j):
            # q written to both partition halves via column-tiled matmul
            # pair (no SBUF->SBUF duplication DMA).
            blk = ts(j, BN)
            nc.gpsimd.dma_start(F1H[:, blk], f1[:, blk])
            pq = stp.tile([BM, 2 * BN], F32, tag="st")
            nc.tensor.matmul(pq[0:HID, 0:BN], lhsT=WQT16[:], rhs=F1H[:, blk],
                             start=True, stop=True, tile_position=(0, 0))
            nc.tensor.matmul(pq[HID:128, 0:BN], lhsT=WQT16[:], rhs=F1H[:, blk],
                             start=True, stop=True, tile_position=(0, HID))
            nc.vector.tensor_scalar_add(Q2[:, blk], pq[:, 0:BN], BQ2[:])

        def emit_k(j):
            blk = ts(j, BN)
            nc.sync.dma_start(F2H[:, blk], f2[:, blk])
            pk = stp.tile([BM, 2 * BN], F32, tag="st")
            nc.tensor.matmul(pk[0:HID, 0:BN], lhsT=WKT16[:], rhs=F2H[:, blk],
                             start=True, stop=True, tile_position=(0, 0))
            nc.tensor.matmul(pk[HID:128, 0:BN], lhsT=WKT16[:], rhs=F2H[:, blk],
                             start=True, stop=True, tile_position=(0, HID))
            nc.vector.tensor_scalar_add(K2[:, blk], pk[:, 0:BN], BK2[:])

        def emit_g(j):
            pg = stp.tile([BM, 2 * BN], F32, tag="st")
            for i, t in enumerate(range(4 * j, 4 * j + 4)):
                nc.tensor.matmul(pg[:, ts(i, OUT)], lhsT=F2H[:, ts(t, BM)],
                                 rhs=WFT16[:], start=True, stop=True)
            nc.vector.tensor_copy(
                GA[:, 4 * j:4 * j + 4, 0:OUT],
                pg[:, 0:4 * OUT].rearrange("p (g o) -> p g o", g=4),
            )

        # ---- phase 1 prologue: just enough for the main loop to start.
        # The rest of the k/G production is interleaved into main block 0
        # so the PE FIFO doesn't serialize all of phase 1 before the first
        # S^T matmul.
        emit_q(0)
        emit_k(0)
        emit_k(1)
        emit_g(0)
        emit_q(1)

        # ---- main loop ----
        for j in range(NB):
            if j + 2 < NB:
                emit_q(j + 2)
            pv = pvp.tile([OUT + 1, BN], F32, tag="pv")
            for t2 in range(MT // 2):
                if j == 0 and t2 % 2 == 0:
                    kb = 2 + t2 // 2
                    if kb < NB:
                        emit_k(kb)
                    gb = 1 + t2 // 2
                    if gb < NB:
                        emit_g(gb)
                st = stp.tile([BM, 2 * BN], F32, tag="st")
                nc.tensor.matmul(st[:, 0:BN], lhsT=K2[:, ts(2 * t2, BM)],
                                 rhs=Q2[:, ts(j, BN)],
                                 start=True, stop=True)
                nc.tensor.matmul(st[:, BN:2 * BN], lhsT=K2[:, ts(2 * t2 + 1, BM)],
                                 rhs=Q2[:, ts(j, BN)],
                                 start=True, stop=True)
                et = etp.tile([BM, 2 * BN], MM_DT, tag="et")
                nc.scalar.activation(et[:], st[:], EXP, scale=0.5,
                                     bias=ESHIFT[:])
                nc.tensor.matmul(pv[:], lhsT=GA[:, 2 * t2, :],
                                 rhs=et[:, 0:BN],
                                 start=(t2 == 0), stop=False)
                nc.tensor.matmul(pv[:], lhsT=GA[:, 2 * t2 + 1, :],
                                 rhs=et[:, BN:2 * BN],
                                 start=False, stop=(t2 == MT // 2 - 1))
            rs = finp.tile([1, BN], F32, tag="rs")
            nc.vector.tensor_copy(rs[:], pv[OUT:, :])
            recip = finp.tile([1, BN], F32, tag="recip")
            nc.vector.reciprocal_approx_fast(recip[:], rs[:])
            recipB = finp.tile([OUT, BN], F32, tag="recipB")
            nc.gpsimd.partition_broadcast(recipB[:], recip[:])
            ob = finp.tile([OUT, BN], F32, tag="ob")
            nc.vector.tensor_mul(ob[:], pv[0:OUT, :], recipB[:])
            nc.vector.tensor_scalar_add(ob[:], ob[:], BF[:])
            nc.gpsimd.dma_start(out[:, ts(j, BN)], ob[:])


_CACHE = {}


def _build():
    if "nc" in _CACHE:
        return _CACHE["nc"]
    nc = bacc.Bacc(None, target_bir_lowering=False, debug=False)
    f1 = nc.dram_tensor("f1", [C, N], MM_DT, kind="ExternalInput")
    f2 = nc.dram_tensor("f2", [C, N], MM_DT, kind="ExternalInput")
    wqT = nc.dram_tensor("wqT", [C, HID], MM_DT, kind="ExternalInput")
    wkT = nc.dram_tensor("wkT", [C, HID], MM_DT, kind="ExternalInput")
    wfT = nc.dram_tensor("wfT", [C, OUT], MM_DT, kind="ExternalInput")
    bq = nc.dram_tensor("bq", [HID, 1], F32, kind="ExternalInput")
    bk = nc.dram_tensor("bk", [HID, 1], F32, kind="ExternalInput")
    bf = nc.dram_tensor("bf", [OUT, 1], F32, kind="ExternalInput")
    out = nc.dram_tensor("out", [OUT, N], F32, kind="ExternalOutput")
    with tile.TileContext(nc) as tc:
        _emit(nc, tc, (f1[:], f2[:], wqT[:], wkT[:], wfT[:],
                       bq[:], bk[:], bf[:], out[:]))
    nc.compile()
    _CACHE["nc"] = nc
    return nc


def _run(inputs, trace=False):
    nc = _build()
    f1 = np.ascontiguousarray(np.asarray(inputs["f1"], np.float32).astype(np.float16))
    f2 = np.ascontiguousarray(np.asarray(inputs["f2"], np.float32).astype(np.float16))
    shared = {
        "wqT": np.ascontiguousarray(np.asarray(inputs["wq"], np.float32).T.astype(np.float16)),
        "wkT": np.ascontiguousarray(np.asarray(inputs["wk"], np.float32).T.astype(np.float16)),
        "wfT": np.ascontiguousarray(np.asarray(inputs["wf"], np.float32).T.astype(np.float16)),
        "bq": np.asarray(inputs["bq"], np.float32).reshape(HID, 1).copy(),
        "bk": np.asarray(inputs["bk"], np.float32).reshape(HID, 1).copy(),
        "bf": np.asarray(inputs["bf"], np.float32).reshape(OUT, 1).copy(),
    }
    in_maps = [
        {"f1": f1[b].reshape(C, N), "f2": f2[b].reshape(C, N), **shared}
        for b in range(B)
    ]
    res = bass_utils.run_bass_kernel_spmd(
        nc, in_maps, core_ids=list(range(B)), trace=trace
    )
    out = np.stack([res.results[b]["out"] for b in range(B)])
    H = int(np.sqrt(N))
    return out.reshape(B, OUT, H, H), res


def kernel(**inputs):
    out, _ = _run(inputs, trace=False)
    return out
